# revision 22
# baseline (speedup 1.0000x reference)
# GAT (2-layer graph attention network) on 8 Trainium2 NeuronCores.
#
# Strategy (dst-sharded, per the sharding hint):
#  - Host: add self-loops (mean edge_attr), assign dst nodes to cores by
#    orig id, per-core sort nodes by (lo_deg, hi_deg) into 49 blocks of 128,
#    pad each block's in-edges to uniform per-block-index slot counts
#    (shared across cores so the SPMD program is identical), precompute
#    per-edge a_e = edge_attr @ (We . att_e) with -100 at pad slots, and
#    remap src ids into table-position space split at 31360 so dma_gather's
#    int16 indices fit.
#  - Device per layer: project features blockwise on PE with
#    [W | U_att_src | V_att_dst] appended (attention scalars land in PSUM
#    for free), write bf16 rows [feat | a_src] to a Shared DRAM table via
#    AllGather; per dst block: dma_gather source rows, compute
#    w = max(exp(z + a_dst), exp(0.2 z + 0.2 a_dst)) (== exp(leaky_relu))
#    on ACT, normalize by a fused reduce, scale rows by alpha (DVE) and
#    segment-sum via identity-lhsT matmuls accumulating in PSUM.
import os
import numpy as np
import ml_dtypes

# ---- problem constants ----
N = 50000
IN_DIM = 256
HID = 128
OUT = 128
HEADS = 2
NCORES = 8
NPC = N // NCORES            # 6250
BLK = 128
NBLK = (NPC + BLK - 1) // BLK  # 49
NPC_PAD = NBLK * BLK         # 6272
NTAB = NCORES * NPC_PAD      # 50176
SPLIT_CORE = 5
SPLIT = SPLIT_CORE * NPC_PAD   # 31360 (< 32768, int16-safe)
SPLIT_ORIG = SPLIT_CORE * NPC  # 31250
PAD_AE = -100.0
ROW1 = 384   # bf16 elems per table1 row: 256 feat + 2 a_src + pad
ROW2 = 256   # bf16 elems per table2 row: 128 feat + 1 a_src + pad
DEN_EPS = 1e-20

bf16 = ml_dtypes.bfloat16

_CACHE = {}


def _host_prep(inputs):
    src = np.asarray(inputs["edge_index"][0], dtype=np.int64)
    dst = np.asarray(inputs["edge_index"][1], dtype=np.int64)
    ea = np.asarray(inputs["edge_attr"], dtype=np.float32)

    cnt = np.bincount(dst, minlength=N).astype(np.float32)
    sums = np.stack(
        [np.bincount(dst, weights=ea[:, c], minlength=N) for c in range(ea.shape[1])],
        axis=1,
    ).astype(np.float32)
    loop_attr = sums / np.maximum(cnt, 1.0)[:, None]
    src_all = np.concatenate([src, np.arange(N, dtype=np.int64)])
    dst_all = np.concatenate([dst, np.arange(N, dtype=np.int64)])
    ea_all = np.concatenate([ea, loop_attr], axis=0).astype(np.float32)

    deg = (cnt + 1).astype(np.int64)
    is_lo = src_all < SPLIT_ORIG
    lo_d = np.bincount(dst_all[is_lo], minlength=N)
    hi_d = deg - lo_d

    pos_of_node = np.empty(N, dtype=np.int64)
    node_of_pos = np.full(NTAB, -1, dtype=np.int64)
    for c in range(NCORES):
        nodes = np.arange(c * NPC, (c + 1) * NPC)
        order = np.lexsort((hi_d[nodes], lo_d[nodes]))
        ranked = nodes[order]
        pos_of_node[ranked] = c * NPC_PAD + np.arange(NPC)
        node_of_pos[c * NPC_PAD : c * NPC_PAD + NPC] = ranked

    K_lo = np.zeros(NBLK, dtype=np.int64)
    K_hi = np.zeros(NBLK, dtype=np.int64)
    for c in range(NCORES):
        for j in range(NBLK):
            nodes = node_of_pos[c * NPC_PAD + j * BLK : c * NPC_PAD + (j + 1) * BLK]
            nodes = nodes[nodes >= 0]
            if len(nodes):
                K_lo[j] = max(K_lo[j], lo_d[nodes].max())
                K_hi[j] = max(K_hi[j], hi_d[nodes].max())
    K = K_lo + K_hi
    slot_off = np.concatenate([[0], np.cumsum(K)]).astype(np.int64)
    lo_off = np.concatenate([[0], np.cumsum(K_lo)]).astype(np.int64)
    hi_off = np.concatenate([[0], np.cumsum(K_hi)]).astype(np.int64)
    S_tot = int(slot_off[-1])

    dpos = pos_of_node[dst_all]
    core_e = dpos // NPC_PAD
    lrank = dpos % NPC_PAD
    blk_e = lrank // BLK
    d_e = lrank % BLK
    order = np.lexsort((np.arange(len(dst_all)), ~is_lo, dpos))
    keys = dpos[order] * 2 + (~is_lo[order]).astype(np.int64)
    new_seg = np.concatenate([[True], keys[1:] != keys[:-1]])
    seg_start_idx = np.nonzero(new_seg)[0]
    seg_id = np.cumsum(new_seg) - 1
    k_within = np.zeros(len(dst_all), dtype=np.int64)
    k_within[order] = np.arange(len(dst_all)) - seg_start_idx[seg_id]
    slot_e = np.where(is_lo, k_within, K_lo[blk_e] + k_within)

    We1 = np.asarray(inputs["We1"], np.float32)
    We2 = np.asarray(inputs["We2"], np.float32)
    ae1vec = np.einsum("dhc,hc->dh", We1.reshape(2, HEADS, HID),
                       np.asarray(inputs["att_e1"], np.float32))
    ae2vec = np.einsum("dhc,hc->dh", We2.reshape(2, 1, OUT),
                       np.asarray(inputs["att_e2"], np.float32))
    ae1_e = ea_all @ ae1vec
    ae2_e = (ea_all @ ae2vec)[:, 0]

    def wrap_idx(flat):
        # logical index i -> sbuf [i % 16, i // 16], replicated to 128 parts
        assert len(flat) % 16 == 0
        arr = flat.reshape(-1, 16).T.astype(np.int16)  # [16, W]
        return np.tile(arr, (8, 1))                    # [128, W]

    per_core = []
    for c in range(NCORES):
        em = np.nonzero(core_e == c)[0]
        bj, de, ke = blk_e[em], d_e[em], k_within[em]
        sl = slot_e[em]
        spos = pos_of_node[src_all[em]]
        elo = is_lo[em]
        idx_lo = np.zeros(int(lo_off[-1]) * BLK, dtype=np.int64)
        idx_hi = np.zeros(int(hi_off[-1]) * BLK, dtype=np.int64)
        idx_lo[(lo_off[bj[elo]] + ke[elo]) * BLK + de[elo]] = spos[elo]
        idx_hi[(hi_off[bj[~elo]] + ke[~elo]) * BLK + de[~elo]] = spos[~elo] - SPLIT
        ae1_arr = np.full((BLK, S_tot, HEADS), PAD_AE, dtype=np.float32)
        ae2_arr = np.full((BLK, S_tot), PAD_AE, dtype=np.float32)
        ae1_arr[de, slot_off[bj] + sl, :] = ae1_e[em]
        ae2_arr[de, slot_off[bj] + sl] = ae2_e[em]
        per_core.append(dict(
            idx_lo=wrap_idx(idx_lo), idx_hi=wrap_idx(idx_hi),
            ae1=ae1_arr.reshape(BLK, S_tot * HEADS).astype(bf16),
            ae2=ae2_arr.astype(bf16),
        ))

    W1 = np.asarray(inputs["W1"], np.float32)
    U1 = np.einsum("ihc,hc->ih", W1.reshape(IN_DIM, HEADS, HID),
                   np.asarray(inputs["att_src1"], np.float32))
    V1 = np.einsum("ihc,hc->ih", W1.reshape(IN_DIM, HEADS, HID),
                   np.asarray(inputs["att_dst1"], np.float32))
    W1ext = np.concatenate([W1, U1, V1], axis=1).astype(bf16)   # [256, 260]
    W2 = np.asarray(inputs["W2"], np.float32)
    U2 = np.einsum("ihc,hc->ih", W2.reshape(IN_DIM, 1, OUT),
                   np.asarray(inputs["att_src2"], np.float32))
    V2 = np.einsum("ihc,hc->ih", W2.reshape(IN_DIM, 1, OUT),
                   np.asarray(inputs["att_dst2"], np.float32))
    W2ext = np.concatenate([W2, U2, V2], axis=1).astype(bf16)   # [256, 130]

    x = np.asarray(inputs["x"], np.float32)
    x_t_slabs = []
    for c in range(NCORES):
        nodes = node_of_pos[c * NPC_PAD : (c + 1) * NPC_PAD]
        xs = np.zeros((NPC_PAD, IN_DIM), np.float32)
        valid = nodes >= 0
        xs[valid] = x[nodes[valid]]
        x_t_slabs.append(np.ascontiguousarray(xs.T).astype(bf16))  # [256, NPC_PAD]

    b1 = np.asarray(inputs["b1"], np.float32)
    b2 = np.asarray(inputs["b2"], np.float32)

    return dict(
        per_core=per_core, K_lo=K_lo, K_hi=K_hi, K=K,
        slot_off=slot_off, lo_off=lo_off, hi_off=hi_off, S_tot=S_tot,
        node_of_pos=node_of_pos, W1ext=W1ext, W2ext=W2ext,
        x_t_slabs=x_t_slabs, b1=b1, b2=b2,
    )


def _build_program(K_lo, K_hi, S_tot, b1, b2):
    MAXBLK = int(os.environ.get("GAT_MAXBLK", str(NBLK)))
    SKIP_L2 = bool(int(os.environ.get("GAT_SKIP_L2", "0")))
    PHA_BLK = int(os.environ.get("GAT_PHA_BLK", str(NBLK)))
    STRIP = int(os.environ.get("GAT_STRIP", "0"))
    import concourse.bacc as bacc
    import concourse.bass as bass
    import concourse.tile as tile
    from concourse import mybir
    from concourse.library_config import mlp as mlp_lib

    f32 = mybir.dt.float32
    bfl = mybir.dt.bfloat16
    i16 = mybir.dt.int16
    Alu = mybir.AluOpType
    Act = mybir.ActivationFunctionType

    K_lo = [int(v) for v in K_lo]
    K_hi = [int(v) for v in K_hi]
    K = [a + b for a, b in zip(K_lo, K_hi)]
    lo_off = np.concatenate([[0], np.cumsum(K_lo)]).astype(int)
    hi_off = np.concatenate([[0], np.cumsum(K_hi)]).astype(int)
    slot_off = np.concatenate([[0], np.cumsum(K)]).astype(int)
    LOW = int(lo_off[-1]) * 8   # idx tile cols (BLK/16 = 8 per plane)
    HIW = int(hi_off[-1]) * 8
    b1_nz = bool(np.any(b1))
    b2_nz = bool(np.any(b2))

    nc = bacc.Bacc("TRN2", target_bir_lowering=False, debug=False,
                   num_devices=NCORES, dynamic_dma_scratch_size=32768,
                   num_swdge_queues=2)

    GMAX_PLANES = 8  # <=1024 descriptors per dma_gather (16KB ring limit)
    qctr = [0]

    def emit_gather(G, p0, nplanes, view, idx_tile, idx_col0, roww):
        # chunk a gather of `nplanes` planes (128 idx each) starting at G
        # slot p0, idx columns idx_col0... ; rotate SWDGE queues
        done = 0
        while done < nplanes:
            pl = min(GMAX_PLANES, nplanes - done)
            q = qctr[0] % 2
            qctr[0] += 1
            nc.gpsimd.dma_gather(
                G[:, p0 + done : p0 + done + pl, :], view,
                idx_tile[:, idx_col0 + done * 8 : idx_col0 + (done + pl) * 8],
                pl * BLK, pl * BLK, roww, elem_step=roww, queue_num=q)
            done += pl

    # ---- I/O ----
    x_t = nc.dram_tensor("x_t", [IN_DIM, NPC_PAD], bfl, kind="ExternalInput")
    w1e = nc.dram_tensor("w1e", [IN_DIM, 260], bfl, kind="ExternalInput")
    w2e = nc.dram_tensor("w2e", [IN_DIM, 130], bfl, kind="ExternalInput")
    idx_lo_d = nc.dram_tensor("idx_lo", [BLK, LOW], i16, kind="ExternalInput")
    idx_hi_d = nc.dram_tensor("idx_hi", [BLK, HIW], i16, kind="ExternalInput")
    ae1_d = nc.dram_tensor("ae1", [BLK, S_tot * HEADS], bfl, kind="ExternalInput")
    ae2_d = nc.dram_tensor("ae2", [BLK, S_tot], bfl, kind="ExternalInput")
    ident_d = nc.dram_tensor("ident", [BLK, BLK], bfl, kind="ExternalInput")
    bias_d = nc.dram_tensor("biases", [1, HEADS * HID + OUT], f32,
                            kind="ExternalInput")
    out_d = nc.dram_tensor("out", [NPC_PAD, OUT], f32, kind="ExternalOutput")

    # ---- internal DRAM ----
    slab1 = nc.dram_tensor("slab1", [NPC_PAD, ROW1], bfl)
    slab2 = nc.dram_tensor("slab2", [NPC_PAD, ROW2], bfl)
    table1 = nc.dram_tensor("table1", [NTAB, ROW1], bfl, addr_space="Shared")
    table2 = nc.dram_tensor("table2", [NTAB, ROW2], bfl, addr_space="Shared")

    groups = [list(range(NCORES))]

    with tile.TileContext(nc) as tc:
        with tc.tile_pool(name="persist", bufs=1) as pp:
            ident = pp.tile([BLK, BLK], bfl)
            nc.sync.dma_start(ident[:], ident_d[:])
            idx_lo_t = pp.tile([BLK, LOW], i16)
            nc.sync.dma_start(idx_lo_t[:], idx_lo_d[:])
            idx_hi_t = pp.tile([BLK, HIW], i16)
            nc.sync.dma_start(idx_hi_t[:], idx_hi_d[:])
            w1et = pp.tile([BLK, 2, 260], bfl)
            nc.sync.dma_start(w1et[:], w1e[:].rearrange("(a k) n -> k a n", k=BLK))
            w2et = pp.tile([BLK, 2, 130], bfl)
            nc.sync.dma_start(w2et[:], w2e[:].rearrange("(a k) n -> k a n", k=BLK))
            adst1 = pp.tile([BLK, NBLK * HEADS], f32)
            adst1s = pp.tile([BLK, NBLK * HEADS], f32)
            adst2 = pp.tile([BLK, NBLK], f32)
            adst2s = pp.tile([BLK, NBLK], f32)
            nc.vector.memset(adst1[:], 0.0)
            nc.vector.memset(adst2[:], 0.0)
            if b1_nz:
                b1t = pp.tile([BLK, HEADS * HID], f32)
                nc.sync.dma_start(
                    b1t[:], bias_d[:, : HEADS * HID].to_broadcast((BLK, HEADS * HID)))
            if b2_nz:
                b2t = pp.tile([BLK, OUT], f32)
                nc.sync.dma_start(
                    b2t[:], bias_d[:, HEADS * HID :].to_broadcast((BLK, OUT)))

            nc.gpsimd.load_library(mlp_lib)

            # ================= phase A: layer-1 projection =================
            with tc.tile_pool(name="phA", bufs=2) as pa, \
                 tc.tile_pool(name="phA_ps", bufs=2, space="PSUM") as pap:
                xt = pa.tile([BLK, 2, NPC_PAD], bfl, tag="xt", bufs=1)
                nc.sync.dma_start(
                    xt[:], x_t[:].rearrange("(a k) n -> k a n", k=BLK))
                for j in range(min(NBLK, PHA_BLK)):
                    ps = pap.tile([BLK, 260], f32)
                    for t in range(2):
                        nc.tensor.matmul(
                            ps[:], xt[:, t, j * BLK : (j + 1) * BLK],
                            w1et[:, t, :], start=(t == 0), stop=(t == 1))
                    st = pa.tile([BLK, ROW1], bfl, tag="stgA")
                    nc.vector.memset(st[:, 258:ROW1], 0.0)
                    nc.vector.tensor_copy(st[:, 0:258], ps[:, 0:258])
                    nc.vector.tensor_copy(
                        adst1[:, j * HEADS : (j + 1) * HEADS], ps[:, 258:260])
                    nc.sync.dma_start(slab1[j * BLK : (j + 1) * BLK, :], st[:])
                nc.vector.tensor_scalar_mul(adst1s[:], adst1[:], 0.2)

            nc.gpsimd.collective_compute(
                "AllGather", Alu.bypass, replica_groups=groups,
                ins=[slab1[:]], outs=[table1[:]])
            tc.strict_bb_all_engine_barrier()

            # ================= layer 1 main loop =================
            t1_lo = table1[0:SPLIT, :]
            t1_hi = table1[SPLIT:NTAB, :]
            with tc.tile_pool(name="main1", bufs=2) as mp, \
                 tc.tile_pool(name="scr1", bufs=2) as sp, \
                 tc.tile_pool(name="ps1", bufs=4, space="PSUM") as psp, \
                 tc.tile_pool(name="ps1b", bufs=2, space="PSUM") as psb:
                for j in range(min(NBLK, MAXBLK)):
                    kl, kh, kk = K_lo[j], K_hi[j], K[j]
                    G = mp.tile([BLK, kk, ROW1], bfl, tag="G")
                    if STRIP & 4:
                        nc.vector.memset(G[:], 0.25)
                    else:
                        if kl:
                            emit_gather(G, 0, kl, t1_lo, idx_lo_t,
                                        lo_off[j] * 8, ROW1)
                        if kh:
                            emit_gather(G, kl, kh, t1_hi, idx_hi_t,
                                        hi_off[j] * 8, ROW1)
                    alpha = sp.tile([BLK, HEADS, kk], f32, tag="alpha")
                    if STRIP & 1:
                        nc.vector.memset(alpha[:], 0.0625)
                    else:
                        aet = sp.tile([BLK, kk * HEADS], bfl, tag="ae")
                        nc.sync.dma_start(
                            aet[:],
                            ae1_d[:, slot_off[j] * HEADS : slot_off[j + 1] * HEADS])
                        z = sp.tile([BLK, kk, HEADS], f32, tag="z")
                        nc.vector.tensor_add(
                            z[:], G[:, :, HEADS * HID : HEADS * HID + HEADS],
                            aet[:].rearrange("p (k h) -> p k h", h=HEADS))
                        w = sp.tile([BLK, HEADS, kk], f32, tag="w")
                        den = sp.tile([BLK, HEADS], f32, tag="den")
                        rden = sp.tile([BLK, HEADS], f32, tag="rden")
                        e1 = sp.tile([BLK, kk], f32, tag="e1")
                        e2 = sp.tile([BLK, kk], f32, tag="e2")
                        for h in range(HEADS):
                            nc.scalar.activation(
                                e1[:], z[:, :, h], Act.Exp,
                                bias=adst1[:, j * HEADS + h : j * HEADS + h + 1],
                                scale=1.0)
                            nc.scalar.activation(
                                e2[:], z[:, :, h], Act.Exp,
                                bias=adst1s[:, j * HEADS + h : j * HEADS + h + 1],
                                scale=0.2)
                            nc.vector.tensor_max(w[:, h, :], e1[:], e2[:])
                            nc.vector.reduce_sum(den[:, h : h + 1], w[:, h, :],
                                                 axis=mybir.AxisListType.X)
                        nc.vector.reciprocal(rden[:], den[:])
                        for h in range(HEADS):
                            nc.vector.tensor_scalar_mul(
                                alpha[:, h, :], w[:, h, :], rden[:, h : h + 1])
                    nps = psp.tile([BLK, HEADS * HID], f32, tag="agg")
                    if STRIP & 2:
                        for k in range(kk):
                            nc.tensor.matmul(nps[:], ident[:], G[:, k, 0:HEADS * HID],
                                             start=(k == 0), stop=(k == kk - 1))
                    else:
                        wG = mp.tile([BLK, kk, HEADS * HID], bfl, tag="wG")
                        for k in range(kk):
                            for h in range(HEADS):
                                nc.vector.tensor_scalar_mul(
                                    wG[:, k, h * HID : (h + 1) * HID],
                                    G[:, k, h * HID : (h + 1) * HID],
                                    alpha[:, h, k : k + 1])
                        for k in range(kk):
                            nc.tensor.matmul(nps[:], ident[:], wG[:, k, :],
                                             start=(k == 0), stop=(k == kk - 1))
                    # ---- ELU + layer-2 projection for this block ----
                    st2 = sp.tile([BLK, ROW2], bfl, tag="stg2")
                    if STRIP & 8:
                        nc.vector.memset(st2[:], 0.125)
                        nc.vector.tensor_copy(adst2[:, j : j + 1], st2[:, 0:1])
                    else:
                        if b1_nz:
                            nc.vector.tensor_add(nps[:], nps[:], b1t[:])
                        tmin = sp.tile([BLK, HEADS * HID], bfl, tag="tmin")
                        nc.vector.tensor_scalar_min(tmin[:], nps[:], 0.0)
                        texp = sp.tile([BLK, HEADS * HID], bfl, tag="texp")
                        nc.scalar.activation(texp[:], tmin[:], Act.Exp)
                        tmax = sp.tile([BLK, HEADS * HID], bfl, tag="tmax")
                        nc.vector.tensor_scalar_max(tmax[:], nps[:], 0.0)
                        hb = sp.tile([BLK, HEADS * HID], bfl, tag="hb")
                        nc.vector.scalar_tensor_tensor(
                            out=hb[:], in0=texp[:], scalar=-1.0, in1=tmax[:],
                            op0=Alu.add, op1=Alu.add)
                        ht = sp.tile([BLK, 2, BLK], bfl, tag="ht")
                        for t in range(2):
                            tp = psb.tile([BLK, BLK], bfl, tag="tp")
                            nc.tensor.transpose(
                                tp[:], hb[:, t * BLK : (t + 1) * BLK], ident[:])
                            nc.vector.tensor_copy(ht[:, t, :], tp[:])
                        p2 = psb.tile([BLK, 130], f32, tag="p2")
                        for t in range(2):
                            nc.tensor.matmul(p2[:], ht[:, t, :], w2et[:, t, :],
                                             start=(t == 0), stop=(t == 1))
                        nc.vector.memset(st2[:, OUT + 1 : ROW2], 0.0)
                        nc.vector.tensor_copy(st2[:, 0 : OUT + 1],
                                              p2[:, 0 : OUT + 1])
                        nc.vector.tensor_copy(adst2[:, j : j + 1],
                                              p2[:, OUT + 1 : OUT + 2])
                    nc.sync.dma_start(slab2[j * BLK : (j + 1) * BLK, :], st2[:])
                nc.vector.tensor_scalar_mul(adst2s[:], adst2[:], 0.2)

            nc.gpsimd.collective_compute(
                "AllGather", Alu.bypass, replica_groups=groups,
                ins=[slab2[:]], outs=[table2[:]])
            tc.strict_bb_all_engine_barrier()

            # ================= layer 2 main loop =================
            t2_lo = table2[0:SPLIT, :]
            t2_hi = table2[SPLIT:NTAB, :]
            with tc.tile_pool(name="main2", bufs=2) as mp, \
                 tc.tile_pool(name="scr2", bufs=2) as sp, \
                 tc.tile_pool(name="ps2", bufs=4, space="PSUM") as psp:
                for j in range(0 if SKIP_L2 else min(NBLK, MAXBLK)):
                    kl, kh, kk = K_lo[j], K_hi[j], K[j]
                    G = mp.tile([BLK, kk, ROW2], bfl, tag="G2")
                    if kl:
                        emit_gather(G, 0, kl, t2_lo, idx_lo_t,
                                    lo_off[j] * 8, ROW2)
                    if kh:
                        emit_gather(G, kl, kh, t2_hi, idx_hi_t,
                                    hi_off[j] * 8, ROW2)
                    aet = sp.tile([BLK, kk], bfl, tag="ae2")
                    nc.sync.dma_start(aet[:], ae2_d[:, slot_off[j] : slot_off[j + 1]])
                    z = sp.tile([BLK, kk], f32, tag="z2")
                    nc.vector.tensor_add(
                        z[:],
                        G[:, :, OUT : OUT + 1].rearrange("p k o -> p (k o)"),
                        aet[:])
                    w = sp.tile([BLK, kk], f32, tag="w2")
                    alpha = sp.tile([BLK, kk], f32, tag="alpha2")
                    den = sp.tile([BLK, 1], f32, tag="den2")
                    rden = sp.tile([BLK, 1], f32, tag="rden2")
                    e1 = sp.tile([BLK, kk], f32, tag="e1b")
                    e2 = sp.tile([BLK, kk], f32, tag="e2b")
                    nc.scalar.activation(e1[:], z[:], Act.Exp,
                                         bias=adst2[:, j : j + 1], scale=1.0)
                    nc.scalar.activation(e2[:], z[:], Act.Exp,
                                         bias=adst2s[:, j : j + 1], scale=0.2)
                    nc.vector.tensor_max(w[:], e1[:], e2[:])
                    nc.vector.reduce_sum(den[:], w[:], axis=mybir.AxisListType.X)
                    nc.vector.reciprocal(rden[:], den[:])
                    nc.vector.tensor_scalar_mul(alpha[:], w[:], rden[:])
                    wG = mp.tile([BLK, kk, OUT], bfl, tag="wG2")
                    for k in range(kk):
                        nc.vector.tensor_scalar_mul(
                            wG[:, k, :], G[:, k, 0:OUT], alpha[:, k : k + 1])
                    nps = psp.tile([BLK, OUT], f32, tag="agg2")
                    for k in range(kk):
                        nc.tensor.matmul(nps[:], ident[:], wG[:, k, :],
                                         start=(k == 0), stop=(k == kk - 1))
                    ob = sp.tile([BLK, OUT], f32, tag="ob")
                    if b2_nz:
                        nc.vector.tensor_add(ob[:], nps[:], b2t[:])
                    else:
                        nc.vector.tensor_copy(ob[:], nps[:])
                    nc.sync.dma_start(out_d[j * BLK : (j + 1) * BLK, :], ob[:])
                first_unwritten = 0 if SKIP_L2 else min(NBLK, MAXBLK)
                if first_unwritten < NBLK:
                    zt = sp.tile([BLK, OUT], f32, tag="zfill")
                    nc.vector.memset(zt[:], 0.0)
                    for j in range(first_unwritten, NBLK):
                        nc.sync.dma_start(out_d[j * BLK : (j + 1) * BLK, :], zt[:])

    nc.compile()
    return nc


def _get_program(prep):
    key = (tuple(int(v) for v in prep["K_lo"]),
           tuple(int(v) for v in prep["K_hi"]),
           bool(np.any(prep["b1"])), bool(np.any(prep["b2"])))
    if key not in _CACHE:
        _CACHE[key] = _build_program(
            prep["K_lo"], prep["K_hi"], prep["S_tot"], prep["b1"], prep["b2"])
    return _CACHE[key]


def kernel(**inputs):
    from concourse.bass_utils import run_bass_kernel_spmd

    prep = _host_prep(inputs)
    nc = _get_program(prep)

    ident_np = np.eye(BLK, dtype=bf16)
    biases = np.concatenate([prep["b1"], prep["b2"]]).reshape(1, -1).astype(np.float32)
    in_maps = []
    for c in range(NCORES):
        pc = prep["per_core"][c]
        in_maps.append({
            "x_t": prep["x_t_slabs"][c],
            "w1e": prep["W1ext"],
            "w2e": prep["W2ext"],
            "idx_lo": pc["idx_lo"],
            "idx_hi": pc["idx_hi"],
            "ae1": pc["ae1"],
            "ae2": pc["ae2"],
            "ident": ident_np,
            "biases": biases,
        })

    res = run_bass_kernel_spmd(nc, in_maps, list(range(NCORES)),
                               trace=bool(int(os.environ.get("GAT_TRACE", "0"))))
    kernel.last_results = res

    out_full = np.zeros((N, OUT), np.float32)
    for c in range(NCORES):
        nodes = prep["node_of_pos"][c * NPC_PAD : (c + 1) * NPC_PAD]
        valid = nodes >= 0
        out_full[nodes[valid]] = res.results[c]["out"][valid]
    return out_full
